# revision 23
# baseline (speedup 1.0000x reference)
"""3-layer GCN encoder on 8 Trainium2 NeuronCores (Bass/Tile).

Strategy (1D node partitioning, cross-layer pipelined halo exchange):
  - dst nodes sharded contiguously across 8 cores (Nc=12500/core).
  - Node rows are split into NKC=4 tile-aligned local chunks; the union of
    chunk k across all cores forms src-bucket k (<=25600 rows, int16-indexable).
  - Phase A computes h = (x @ W) * out_norm tile by tile per chunk and fires
    an AllGather per chunk (fp16 rows -> h_full[l][k] on every core).
  - Phase C: per supertile (st_tiles dst tiles), per bucket: messages gathered
    from h_full via gpsimd.dma_gather (int16 indices), segment-summed into
    per-tile PSUM accumulators via one-hot S-matrix matmuls (S built on DVE
    with is_equal vs iota).  Epilogue per dst tile: relu(psum*in_norm + z)
    where z = h*out_norm*in_norm + b is precomputed in phase A.
  - Cross-layer overlap: as soon as phase C finishes the dst tiles of chunk k,
    phase A of layer l+1 runs for chunk k and its AllGather is issued — the
    collectives for layer l+1 execute concurrently with the rest of layer l's
    phase C, so no compute engine ever waits on the interconnect (except the
    very first chunk of layer 0).

Host-side preprocessing (numpy) builds a uniform SPMD plan: per-(tile,
bucket) segment sizes are the max across cores so all 8 cores execute an
identical instruction stream; per-core index/dst-local arrays are data.
"""

import os
import numpy as np

P = 128
N_CORES = 8
TBL_W = 128  # gather table row width (fp16 -> 256B rows)

_BUILD_CACHE = {}

# --------------------------------------------------------------------------
# Host preprocessing: build the uniform aggregation plan
# --------------------------------------------------------------------------


class Plan:
    pass


def make_plan(N, src, dst, n_cores, st_tiles=5, nkc=4):
    E = src.shape[0]
    Nc = N // n_cores
    assert Nc * n_cores == N
    n_tiles = -(-Nc // P)

    # tile-aligned local chunk boundaries (chunk k rows of every core form
    # src bucket k); tiles-per-chunk must be a multiple of st_tiles so that
    # supertiles never straddle a chunk boundary.
    tpc = -(-n_tiles // nkc)
    tpc = -(-tpc // st_tiles) * st_tiles
    CK = [min(Nc, k * tpc * P) for k in range(nkc)] + [Nc]
    sz = [CK[k + 1] - CK[k] for k in range(nkc)]
    NB = nkc
    assert max(s * n_cores for s in sz) < 32768

    owner = dst // Nc
    r = dst - owner * Nc
    tl = r // P
    dl = (r % P).astype(np.int32)
    sc = src // Nc
    sr = src - sc * Nc
    bk = np.searchsorted(CK, sr, side="right") - 1
    np.clip(bk, 0, NB - 1, out=bk)
    szv = np.array(sz, dtype=np.int64)
    ck0 = np.array(CK[:-1], dtype=np.int64)
    sl = (sc * szv[bk] + (sr - ck0[bk])).astype(np.int32)

    key = (owner.astype(np.int64) * n_tiles + tl) * NB + bk
    order = np.lexsort((src, key))
    key_s = key[order]
    sl_s = sl[order]
    dl_s = dl[order]
    tl_s = tl[order].astype(np.int32)

    n_groups = n_cores * n_tiles * NB
    counts = np.bincount(key, minlength=n_groups).reshape(n_cores, n_tiles, NB)
    maxc = counts.max(axis=0)  # [n_tiles, NB]

    # supertile tile-ranges
    sts = [(i, min(i + st_tiles, n_tiles)) for i in range(0, n_tiles, st_tiles)]
    n_st = len(sts)

    # layout: per supertile, per bucket one gather call; segments (t,b)
    # packed unaligned inside the call; call positions chunk(128)-aligned.
    st_infos = []
    gpos = 0
    icol = 0
    npair = 0
    seg_base = np.zeros((n_tiles, NB), dtype=np.int64)
    for si, (t0, t1) in enumerate(sts):
        info = Plan()
        info.t0, info.t1 = t0, t1
        info.calls = []
        info.gpos0 = gpos
        info.icol0 = icol
        st_chunk0 = gpos // P
        for b in range(NB):
            psum_b = int(maxc[t0:t1, b].sum())
            nch = max(1, -(-psum_b // P))
            off = 0
            for t in range(t0, t1):
                seg_base[t, b] = gpos + off
                off += int(maxc[t, b])
            call = Plan()
            call.b = b
            call.nrows = sz[b] * n_cores
            call.cb0 = (gpos // P) - st_chunk0
            call.nch = nch
            call.lcol0 = icol - info.icol0
            call.ncols = nch * P // 16
            info.calls.append(call)
            gpos += nch * P
            icol += nch * P // 16
        info.nchunks = (gpos // P) - st_chunk0
        info.st_chunk0 = st_chunk0
        info.idx_cols = icol - info.icol0
        st_infos.append(info)

    gpos_tot = gpos
    idx_cols_tot = icol

    # pair enumeration: bucket-major within each supertile so the pair ids
    # of one (st,b) gather call are contiguous (-> one batched S build)
    pair_list = []  # (gq, t)
    tiles_by_st = [[] for _ in range(n_st)]
    maxq = 1
    for si, (t0, t1) in enumerate(sts):
        info = st_infos[si]
        bytile = {t: {"t": t, "pairs_by_b": {}, "npr_tot": 0}
                  for t in range(t0, t1)}
        for b in range(NB):
            call = info.calls[b]
            call.pr0 = npair
            for t in range(t0, t1):
                if maxc[t, b] == 0:
                    continue
                p0 = int(seg_base[t, b])
                p1 = p0 + int(maxc[t, b])
                qa = p0 // P
                qb = -(-p1 // P)
                plist = []
                for gq in range(qa, qb):
                    pair_list.append((gq, t))
                    plist.append((gq - info.st_chunk0 - call.cb0, npair))
                    npair += 1
                bytile[t]["pairs_by_b"][b] = plist
                bytile[t]["npr_tot"] += qb - qa
            call.npr = npair - call.pr0
            maxq = max(maxq, call.npr)
        tiles_by_st[si] = [bytile[t] for t in range(t0, t1)]

    n_pairs_tot = npair
    max_idx_cols = max(i.idx_cols for i in st_infos)
    max_call_nch = max(c.nch for i in st_infos for c in i.calls)

    # ---------------- per-core data arrays ----------------
    grp_starts = np.zeros(n_groups + 1, dtype=np.int64)
    np.cumsum(counts.reshape(-1), out=grp_starts[1:])
    rank = np.arange(E, dtype=np.int64) - grp_starts[key_s]

    seg_base_flat = seg_base.reshape(-1)
    tb_of_key = key_s % (n_tiles * NB)
    gpos_of_edge = seg_base_flat[tb_of_key] + rank
    core_of_edge = key_s // (n_tiles * NB)

    idx_flat = np.zeros((n_cores, gpos_tot), dtype=np.int16)
    tile_of_pos = np.full((n_cores, gpos_tot), -1, dtype=np.int16)
    dl_of_pos = np.full((n_cores, gpos_tot), -1, dtype=np.int16)
    idx_flat[core_of_edge, gpos_of_edge] = sl_s.astype(np.int16)
    tile_of_pos[core_of_edge, gpos_of_edge] = tl_s.astype(np.int16)
    dl_of_pos[core_of_edge, gpos_of_edge] = dl_s.astype(np.int16)

    # trailing pads of each call become -1 (gather skips them; stale SBUF
    # data is masked by dstl == -1).  Valid count must be identical across
    # cores (uniform SPMD), so pad up to the max last-valid over cores.
    valid = tile_of_pos >= 0
    for info in st_infos:
        for call in info.calls:
            g0 = (info.st_chunk0 + call.cb0) * P
            g1 = g0 + call.nch * P
            seg = valid[:, g0:g1]
            nz = np.where(seg.any(axis=0))[0]
            nval = int(nz[-1]) + 1 if len(nz) else 1
            call.nvalid = nval
            idx_flat[:, g0 + nval:g1] = -1

    # wrap indices: position i -> [i % 16, i // 16], replicated to 128 rows
    idxw = idx_flat.reshape(n_cores, -1, 16).transpose(0, 2, 1)  # [M,16,cols]
    idxw = np.tile(idxw, (1, 8, 1)).copy()  # [M,128,cols]

    # dst-local per pair: [M, 128, n_pairs]
    dstl = np.full((n_cores, P, n_pairs_tot), -1.0, dtype=np.float16)
    for pi, (gq, t) in enumerate(pair_list):
        s0 = gq * P
        blk_t = tile_of_pos[:, s0:s0 + P]
        blk_d = dl_of_pos[:, s0:s0 + P]
        dstl[:, :, pi] = np.where(blk_t == t, blk_d, -1).astype(np.float16)

    plan = Plan()
    plan.N, plan.E, plan.Nc = N, E, Nc
    plan.n_tiles, plan.NB = n_tiles, NB
    plan.CK, plan.sz, plan.tpc = CK, sz, tpc
    plan.st_tiles = st_tiles
    plan.sts, plan.st_infos = sts, st_infos
    plan.tiles_by_st = tiles_by_st
    plan.n_pairs_tot = n_pairs_tot
    plan.idx_cols_tot = idx_cols_tot
    plan.maxq = maxq
    plan.max_idx_cols = max_idx_cols
    plan.max_call_nch = max_call_nch
    plan.idxw, plan.dstl = idxw, dstl
    plan.gpos_tot = gpos_tot
    return plan


# --------------------------------------------------------------------------
# Device kernel builder
# --------------------------------------------------------------------------


def build_kernel(plan, douts):
    from concourse import bass, bacc, tile, mybir

    f32 = mybir.dt.float32
    f16 = mybir.dt.float16
    i16 = mybir.dt.int16

    N, Nc, n_tiles, NB = plan.N, plan.Nc, plan.n_tiles, plan.NB
    NT128 = n_tiles * P
    NKC = NB
    tpc = plan.tpc
    single_packet = os.environ.get("GCN_SP", "0") == "1"

    nc = bacc.Bacc("TRN2", target_bir_lowering=False, debug=False,
                   num_devices=N_CORES, num_swdge_queues=4)

    xc_d = nc.dram_tensor("xc", [NT128, P], f16, kind="ExternalInput")
    W_d = [nc.dram_tensor(f"W{l}", [P, douts[l]], f16, kind="ExternalInput")
           for l in range(3)]
    B_d = [nc.dram_tensor(f"B{l}", [P, douts[l]], f32, kind="ExternalInput")
           for l in range(3)]
    onorm_d = nc.dram_tensor("onorm", [P, n_tiles], f32, kind="ExternalInput")
    inorm_d = nc.dram_tensor("inorm", [P, n_tiles], f32, kind="ExternalInput")
    iota_d = nc.dram_tensor("iota", [P, plan.maxq * P], f16,
                            kind="ExternalInput")
    ident_d = nc.dram_tensor("ident", [P, P], f16, kind="ExternalInput")
    idxw_d = nc.dram_tensor("idxw", [P, plan.idx_cols_tot], i16,
                            kind="ExternalInput")
    dstl_d = nc.dram_tensor("dstl", [P, plan.n_pairs_tot], f16,
                            kind="ExternalInput")
    out_d = nc.dram_tensor("out", [Nc, douts[2]], f32, kind="ExternalOutput")

    with tile.TileContext(nc) as tc:
        with tc.tile_pool(name="const", bufs=1) as cp, \
             tc.tile_pool(name="msgp", bufs=10) as mp, \
             tc.tile_pool(name="idxp", bufs=6) as ip, \
             tc.tile_pool(name="sp", bufs=3) as sp, \
             tc.tile_pool(name="ep", bufs=4) as ep, \
             tc.tile_pool(name="xtp", bufs=3) as xtp, \
             tc.tile_pool(name="psA", bufs=1, space="PSUM") as psA, \
             tc.tile_pool(name="psC", bufs=6, space="PSUM") as psC, \
             tc.tile_pool(name="dram", bufs=2, space="DRAM") as dr:

            # resident tiles
            x_sb = cp.tile([P, NT128], f16, tag="x")
            hs_sb = cp.tile([P, NT128], f16, tag="hs")
            z_sb = cp.tile([P, NT128], f32, tag="z")
            W_sb = [cp.tile([P, douts[l]], f16, tag=f"W{l}", name=f"W{l}_sb")
                    for l in range(3)]
            B_sb = [cp.tile([P, douts[l]], f32, tag=f"B{l}", name=f"B{l}_sb")
                    for l in range(3)]
            on_sb = cp.tile([P, n_tiles], f32, tag="on")
            in_sb = cp.tile([P, n_tiles], f32, tag="in")
            oin_sb = cp.tile([P, n_tiles], f32, tag="oin")
            iota_sb = cp.tile([P, plan.maxq * P], f16, tag="iota")
            id_sb = cp.tile([P, P], f16, tag="ident")
            dstl_sb = cp.tile([P, plan.n_pairs_tot], f16, tag="dstl")

            for l in range(3):
                nc.sync.dma_start(W_sb[l][:], W_d[l].ap())
                nc.sync.dma_start(B_sb[l][:], B_d[l].ap())
            nc.sync.dma_start(on_sb[:], onorm_d.ap())
            nc.sync.dma_start(in_sb[:], inorm_d.ap())
            nc.sync.dma_start(iota_sb[:], iota_d.ap())
            nc.sync.dma_start(id_sb[:], ident_d.ap())
            nc.sync.dma_start(dstl_sb[:], dstl_d.ap())
            nc.sync.dma_start(
                x_sb[:].rearrange("p (t f) -> p t f", f=P),
                xc_d.ap().rearrange("(t p) f -> p t f", p=P))
            nc.vector.tensor_tensor(oin_sb[:], on_sb[:], in_sb[:],
                                    mybir.AluOpType.mult)

            h_full = {}  # (l, k) -> dram tile

            # zero all msg pool slots once: trailing gather positions are
            # skipped (idx -1) and masked by S==0, but 0*NaN from
            # uninitialized SBUF would still poison PSUM.
            for mi in range(10):
                mz = mp.tile([P, plan.max_call_nch * P], f16, tag="msg",
                             name=f"msgz{mi}")
                nc.gpsimd.memset(mz[:], 0)

            def phase_a_chunk(l, k):
                d = douts[l]
                rows_k = plan.sz[k]
                with nc.named_scope(f"L{l}_A{k}"):
                    ag_in = dr.tile([rows_k, TBL_W], f16, tag=f"agin{k}",
                                    name=f"agin_{l}_{k}")
                    hf = dr.tile([rows_k * N_CORES, TBL_W], f16,
                                 tag=f"hfull{k}", name=f"hfull_{l}_{k}",
                                 addr_space="Shared")
                    h_full[(l, k)] = hf
                    t0k = k * tpc
                    t1k = min(n_tiles, -(-plan.CK[k + 1] // P))
                    for t in range(t0k, t1k):
                        rows = min(P, plan.CK[k + 1] - t * P)
                        pa_t = psA.tile([P, P], f16, tag="pat", name="pa_t")
                        nc.tensor.transpose(pa_t[:],
                                            x_sb[:, t * P:(t + 1) * P],
                                            id_sb[:])
                        xT = xtp.tile([P, P], f16, tag="xT")
                        nc.scalar.copy(xT[:], pa_t[:])
                        h_ps = psA.tile([P, d], f32, tag="pah", name="h_ps")
                        nc.tensor.matmul(h_ps[:], lhsT=xT[:],
                                         rhs=W_sb[l][:],
                                         start=True, stop=True)
                        nc.scalar.activation(
                            hs_sb[:, t * P:t * P + d], h_ps[:],
                            mybir.ActivationFunctionType.Copy,
                            scale=on_sb[:, t:t + 1])
                        nc.scalar.activation(
                            z_sb[:, t * P:t * P + d],
                            hs_sb[:, t * P:t * P + d],
                            mybir.ActivationFunctionType.Copy,
                            scale=in_sb[:, t:t + 1])
                        nc.vector.tensor_tensor(
                            z_sb[:, t * P:t * P + d],
                            z_sb[:, t * P:t * P + d],
                            B_sb[l][:], mybir.AluOpType.add)
                        nc.sync.dma_start(
                            ag_in[(t - t0k) * P:(t - t0k) * P + rows, :],
                            hs_sb[:rows, t * P:(t + 1) * P])
                    nc.gpsimd.collective_compute(
                        "AllGather", bass.mybir.AluOpType.bypass,
                        replica_groups=[list(range(N_CORES))],
                        ins=[ag_in.opt()],
                        outs=[hf.opt()])

            gq_rr = 0

            def issue_gathers(l, si, b):
                nonlocal gq_rr
                info = plan.st_infos[si]
                st_msgs = msgs_of.setdefault((l, si), {})
                if b == 0:
                    idxt = ip.tile([P, plan.max_idx_cols], i16, tag="idx",
                                   name="idxt")
                    nc.sync.dma_start(
                        idxt[:, :info.idx_cols],
                        idxw_d.ap()[:, info.icol0:info.icol0
                                    + info.idx_cols])
                    st_msgs["idxt"] = idxt
                idxt = st_msgs["idxt"]
                call = info.calls[b]
                msg = mp.tile([P, plan.max_call_nch * P], f16, tag="msg",
                              name="msg")
                st_msgs[call.b] = msg
                oap = msg[:, :call.nch * P]
                oap = oap.rearrange("p (q e) -> p q e", e=P)
                nc.gpsimd.dma_gather(
                    out_ap=oap,
                    in_ap=h_full[(l, call.b)][0:call.nrows, :],
                    idxs_ap=idxt[:, call.lcol0:call.lcol0
                                 + call.ncols],
                    num_idxs=call.nch * P,
                    num_idxs_reg=call.nvalid,
                    elem_size=TBL_W,
                    single_packet=single_packet,
                    queue_num=gq_rr % 4)
                gq_rr += 1

            msgs_of = {}

            def phase_c_supertile(l, si):
                d = douts[l]
                info = plan.st_infos[si]
                with nc.named_scope(f"L{l}_C{si}"):
                    msgs = msgs_of.pop((l, si))

                    tinfos = plan.tiles_by_st[si]
                    ps_of = {}
                    done_of = {}
                    for tinfo in tinfos:
                        ps_of[tinfo["t"]] = psC.tile([P, d], f32, tag="pc",
                                                     name="ps")
                        done_of[tinfo["t"]] = 0
                    for b in range(NB):
                        call = info.calls[b]
                        if call.npr == 0:
                            continue
                        S = sp.tile([P, plan.maxq * P], f16, tag="S")
                        nc.vector.tensor_tensor(
                            S[:, :call.npr * P].rearrange(
                                "p (q e) -> p q e", e=P),
                            iota_sb[:, :call.npr * P].rearrange(
                                "p (q e) -> p q e", e=P),
                            dstl_sb[:, call.pr0:call.pr0
                                    + call.npr].broadcast_to(
                                (P, call.npr, P)),
                            mybir.AluOpType.is_equal)
                        msg = msgs[b]
                        for tinfo in tinfos:
                            t = tinfo["t"]
                            plist = tinfo["pairs_by_b"].get(b)
                            if not plist:
                                continue
                            ps = ps_of[t]
                            for (q_local, pr) in plist:
                                ki = pr - call.pr0
                                nc.tensor.matmul(
                                    ps,
                                    lhsT=S[:, ki * P:(ki + 1) * P],
                                    rhs=msg[:, q_local * P:q_local * P + d],
                                    start=(done_of[t] == 0),
                                    stop=(done_of[t]
                                          == tinfo["npr_tot"] - 1))
                                done_of[t] += 1

                    # epilogue per tile
                    for tinfo in tinfos:
                        t = tinfo["t"]
                        rows = min(P, Nc - t * P)
                        ps = ps_of[t]
                        t2 = ep.tile([P, d], f32, tag="t2")
                        nc.scalar.activation(
                            t2[:], ps,
                            mybir.ActivationFunctionType.Copy,
                            scale=in_sb[:, t:t + 1])
                        t3 = ep.tile([P, d], f32, tag="t3")
                        nc.vector.tensor_tensor(
                            t3[:], t2[:], z_sb[:, t * P:t * P + d],
                            mybir.AluOpType.add)
                        if l < 2:
                            nc.scalar.activation(
                                x_sb[:, t * P:t * P + d], t3[:],
                                mybir.ActivationFunctionType.Relu)
                        else:
                            nc.sync.dma_start(
                                out_d.ap()[t * P:t * P + rows, :],
                                t3[:rows, :])

            # layer-0 phase A fully upfront (cheap; AGs pipeline on the
            # collective cores)
            pipe = os.environ.get("GCN_PIPE", "1") == "1"
            n_st = len(plan.sts)

            def issue_pair(l, p):
                for b in range(NB):
                    for si in (p, p + 1):
                        if si < n_st:
                            issue_gathers(l, si, b)

            for k in range(NKC):
                phase_a_chunk(0, k)
            for l in range(3):
                next_k = 0
                issue_pair(l, 0)
                for si in range(n_st):
                    if si % 2 == 0 and si + 2 < n_st:
                        issue_pair(l, si + 2)
                    phase_c_supertile(l, si)
                    tiles_done = min(n_tiles, (si + 1) * plan.st_tiles)
                    while (pipe and l < 2 and next_k < NKC
                           and tiles_done >= min(n_tiles,
                                                 -(-plan.CK[next_k + 1]
                                                   // P))):
                        phase_a_chunk(l + 1, next_k)
                        next_k += 1
                if not pipe and l < 2:
                    for k in range(NKC):
                        phase_a_chunk(l + 1, k)

    nc.compile()
    return nc


# --------------------------------------------------------------------------
# Entry point
# --------------------------------------------------------------------------

LAST_EXEC_NS = None
LAST_RES = None


def kernel(feat, src, dst, W1, b1, W2, b2, W3, b3):
    global LAST_EXEC_NS, LAST_RES
    from concourse.bass_utils import run_bass_kernel_spmd

    feat = np.asarray(feat, dtype=np.float32)
    src = np.asarray(src, dtype=np.int32)
    dst = np.asarray(dst, dtype=np.int32)
    Ws = [np.asarray(w, np.float32) for w in (W1, W2, W3)]
    bs = [np.asarray(b, np.float32) for b in (b1, b2, b3)]

    N, F = feat.shape
    douts = [w.shape[1] for w in Ws]

    st_tiles = int(os.environ.get("GCN_ST_TILES", "5"))
    nkc = int(os.environ.get("GCN_NKC", "4"))

    import hashlib
    h = hashlib.sha1()
    h.update(src.tobytes())
    h.update(dst.tobytes())
    key = (N, F, tuple(douts), h.hexdigest(), st_tiles, nkc,
           os.environ.get("GCN_PIPE", "1"), os.environ.get("GCN_SP", "0"))

    if key in _BUILD_CACHE:
        nc, plan = _BUILD_CACHE[key]
    else:
        plan = make_plan(N, src, dst, N_CORES, st_tiles, nkc)
        nc = build_kernel(plan, douts)
        _BUILD_CACHE.clear()
        _BUILD_CACHE[key] = (nc, plan)

    Nc, n_tiles = plan.Nc, plan.n_tiles

    # norms (degrees include self-loops)
    deg_out = np.bincount(src, minlength=N).astype(np.float32) + 1.0
    deg_in = np.bincount(dst, minlength=N).astype(np.float32) + 1.0
    out_norm = (1.0 / np.sqrt(deg_out)).astype(np.float32)
    in_norm = (1.0 / np.sqrt(deg_in)).astype(np.float32)

    NT128 = n_tiles * P
    iota = np.tile(np.arange(P, dtype=np.float16), plan.maxq)
    iota = np.broadcast_to(iota, (P, plan.maxq * P)).copy()
    ident = np.eye(P, dtype=np.float16)

    in_maps = []
    for c in range(N_CORES):
        xc = np.zeros((NT128, P), np.float16)
        xc[:Nc] = feat[c * Nc:(c + 1) * Nc]
        onorm = np.zeros((P, n_tiles), np.float32)
        inorm = np.zeros((P, n_tiles), np.float32)
        on_pad = np.zeros(NT128, np.float32)
        on_pad[:Nc] = out_norm[c * Nc:(c + 1) * Nc]
        in_pad = np.zeros(NT128, np.float32)
        in_pad[:Nc] = in_norm[c * Nc:(c + 1) * Nc]
        onorm[:, :] = on_pad.reshape(n_tiles, P).T
        inorm[:, :] = in_pad.reshape(n_tiles, P).T
        m = {
            "xc": xc,
            "onorm": onorm,
            "inorm": inorm,
            "iota": iota,
            "ident": ident,
            "idxw": plan.idxw[c],
            "dstl": plan.dstl[c],
        }
        for l in range(3):
            m[f"W{l}"] = Ws[l].astype(np.float16)
            m[f"B{l}"] = np.broadcast_to(bs[l], (P, douts[l])).copy()
        in_maps.append(m)

    trace = os.environ.get("GCN_TRACE", "0") == "1"
    res = run_bass_kernel_spmd(nc, in_maps, core_ids=list(range(N_CORES)),
                               trace=trace)
    LAST_EXEC_NS = res.exec_time_ns
    LAST_RES = res
    out = np.concatenate([res.results[c]["out"] for c in range(N_CORES)],
                         axis=0)
    return out[:N].astype(np.float32)
